# revision 91
# baseline (speedup 1.0000x reference)
"""Trainium2 Bass kernel for DualHeterogeneousTransformer (returns out[:, 0] only).

Algebraic reduction (reference returns only query row 0):
  q      = (x[:,0,:] + pos_e[0]) @ We_q^T + be_q                      [B,D]
  s_e[b,k] = xp[b,k,:].qk_e[b] + qdot_e[b]          (k<64, pos_e folded into xp)
  s_e[b,64] = x0p[b].v64 + c64 + qdot_e[b]          (mask token, folded weights)
  s_r[b,k] = r[b].qk_r[b] + pos_r[k].qk_r[b] + qdot_r[b]
  p = exp(s); C_e^T = sum_k p_e[k] xp_k^T + p64*Pe64^T
  C_r^T = sar * r^T + pos_r^T @ p_r^T
  outT = We_v^T-chunks @ C_e^T + Wr_v^T-chunks @ C_r^T      (unnormalized)
  host: out = (outT^T + sae*be_v + sar*br_v) / (sae + sar)

Everything streamed/computed in bf16 (except exp/score accumulators in f32);
x is read from HBM exactly once as bf16 (16MB/core).  The weighted-value
accumulation runs on the PE via diag(p_k) stationary matmuls accumulating
C^T in PSUM; dot-products are fused mult+accum ops split across Pool/DVE.
"""

import os
import sys

import numpy as np

for _p in ("/opt/trn_rl_repo", "/root/.axon_site/_ro/trn_rl_repo"):
    if os.path.isdir(_p) and _p not in sys.path:
        sys.path.insert(0, _p)

import concourse.bass as bass
import concourse.bacc as bacc
import concourse.mybir as mybir
from concourse import tile
from concourse.bass_utils import run_bass_kernel_spmd

B, L, D = 2048, 64, 512
NCORES = 8
BS = B // NCORES          # 256 rows per core
P = 128                   # partition tile of batch rows
NT = BS // P              # 2 batch tiles per core
KC = 16                   # keys per streamed x chunk
NCHUNK = L // KC          # 8 chunks per batch tile
DC = D // P               # 4 contraction chunks of 128
SCALE = float(1.0 / np.sqrt(D))
F32 = mybir.dt.float32
BF16 = mybir.dt.bfloat16
ALU = mybir.AluOpType
ACTF = mybir.ActivationFunctionType
AX = mybir.AxisListType

# per-chunk count of fused dots on DVE (rest on Pool/gpsimd)
DOT_DVE = [8, 9, 12, 12]
# pair-multiplies moved from Pool to DVE tensor_tensor, per chunk
DVE_MULT = [0, 0, 0, 0]
# per-chunk diag-build engine patterns (D=DVE, P=Pool, A=ACT)
DIAG_PATTERNS = ["PPPPDPPPDPPPDPPP", "PPPPDPPPDPPPDPPP",
                 "PPPPDPPPDPPPDPPP", "PPPPDPPPDPPPDPPP"]
# chunk indices (within a tile) whose DMA rides the gpsimd ring instead of SP
POOL_CHUNKS = (2,)
N_WARM = 6


def build_nc():
    nc = bacc.Bacc("TRN2", target_bir_lowering=False, debug=False)

    xp_d = nc.dram_tensor("xp16", [BS, L, D], BF16, kind="ExternalInput")
    x0_d = nc.dram_tensor("x0p16", [BS, D], BF16, kind="ExternalInput")
    r_d = nc.dram_tensor("r16", [BS, D], BF16, kind="ExternalInput")
    # q-chain augmented weights: rows 0..D-1 = W, row D = ones-row consts
    # cols: [0:D]=qk, D=mask-dot col, D+1=qdot col   (entity); rel: D+1 wide
    wqe_d = nc.dram_tensor("wq_e", [P, DC, D + 2 + L + 1], BF16, kind="ExternalInput")
    wqe1_d = nc.dram_tensor("wq_e1", [1, D + 2 + L + 1], BF16, kind="ExternalInput")
    wqr_d = nc.dram_tensor("wq_r", [P, DC, D], BF16, kind="ExternalInput")
    wqr1_d = nc.dram_tensor("wq_r1", [1, D], BF16, kind="ExternalInput")
    # output projection weights, chunked: [p, dc, e] = W_v[e, dc*128+p]
    wev_d = nc.dram_tensor("wev", [P, DC, D], BF16, kind="ExternalInput")
    wrv_d = nc.dram_tensor("wrv", [P, DC, D], BF16, kind="ExternalInput")
    prR_d = nc.dram_tensor("prR", [L, D], BF16, kind="ExternalInput")
    pe64_d = nc.dram_tensor("pe64", [1, D], BF16, kind="ExternalInput")
    id_d = nc.dram_tensor("ident16", [P, P], BF16, kind="ExternalInput")

    outT_d = nc.dram_tensor("outT", [NT, P, DC, P], BF16, kind="ExternalOutput")
    stats_d = nc.dram_tensor("stats", [BS, 2], F32, kind="ExternalOutput")

    with tile.TileContext(nc) as tc:
        with (
            tc.tile_pool(name="const", bufs=1) as const,
            tc.tile_pool(name="work", bufs=2) as work,
            tc.tile_pool(name="psum", bufs=4, space="PSUM") as psum,
        ):
            # PE warmup so the PE clock is fully ramped by first real matmul
            warm = work.tile([P, P], BF16, tag="warm")
            nc.vector.memset(warm[:], 0.0)
            ones1 = const.tile([1, P], BF16, tag="ones1")
            nc.vector.memset(ones1[:], 1.0)
            ps_w = psum.tile([P, D], F32, tag="ps")
            for wi in range(N_WARM):
                nc.tensor.matmul(ps_w[:, 0:P], warm[:], warm[:],
                                 start=(wi == 0), stop=(wi == N_WARM - 1))

            xpool = tc.alloc_tile_pool(name="xchunk", bufs=7)
            junkpool = tc.alloc_tile_pool(name="junk", bufs=30)
            diagpool = tc.alloc_tile_pool(name="diag", bufs=20)
            tailp = tc.alloc_tile_pool(name="tail", bufs=2)

            # head DMAs: x0p rows (q-chain input), identity, entity q weights
            x0_tiles, r_tiles = [], []
            for ts in range(NT):
                rows = slice(ts * P, (ts + 1) * P)
                x0_t = work.tile([P, D], BF16, tag="x0")
                nc.sync.dma_start(x0_t[:], x0_d[rows, :])
                x0_tiles.append(x0_t)
            ident = const.tile([P, P], BF16, tag="ident")
            nc.gpsimd.dma_start(ident[:], id_d[:])
            wqe = const.tile([P, DC, D + 2 + L + 1], BF16, tag="wqe")
            nc.gpsimd.dma_start(wqe[:], wqe_d[:])
            wqe1 = const.tile([1, D + 2 + L + 1], BF16, tag="wqe1")
            nc.gpsimd.dma_start(wqe1[:], wqe1_d[:])

            wqr = const.tile([P, DC, D], BF16, tag="wqr")
            nc.scalar.dma_start(wqr[:], wqr_d[:])
            wqr1 = const.tile([1, D], BF16, tag="wqr1")
            nc.scalar.dma_start(wqr1[:], wqr1_d[:])
            pe64 = const.tile([1, D], BF16, tag="pe64")
            nc.scalar.dma_start(pe64[:], pe64_d[:])
            for ts in range(NT):
                rows = slice(ts * P, (ts + 1) * P)
                r_t = work.tile([P, D], BF16, tag="r", name=f"r_{ts}")
                nc.scalar.dma_start(r_t[:], r_d[rows, :])
                r_tiles.append(r_t)
            prR = const.tile([L, D], BF16, tag="prR")
            nc.scalar.dma_start(prR[:], prR_d[:])
            # late consts (tail-only), DMA'd from a stream hook on SP
            wev = const.tile([P, DC, D], BF16, tag="wev")
            wrv = const.tile([P, DC, D], BF16, tag="wrv")

            from types import SimpleNamespace

            def mchain_e(ts):
                st = SimpleNamespace()
                st.rows = slice(ts * P, (ts + 1) * P)
                x0_sb = x0_tiles[ts]

                # transpose x0p -> x0T chunks [128d, 128b]
                st.x0T = work.tile([P, DC, P], BF16, tag="x0T")
                ps_x0 = psum.tile([P, DC, P], BF16, tag="ps")
                for kc in range(DC):
                    nc.tensor.transpose(
                        ps_x0[:, kc, :], x0_sb[:, kc * P:(kc + 1) * P], ident[:]
                    )
                    nc.vector.tensor_copy(st.x0T[:, kc, :], ps_x0[:, kc, :])

                # entity q-chain: qk_e cols [0:D], mask-dot col D, qdot col D+1
                XW = D + 2 + L + 1
                ps_qe = psum.tile([P, D], F32, tag="ps")
                ps_qe2 = psum.tile([P, L + 3], F32, tag="ps")
                for kc in range(DC):
                    nc.tensor.matmul(ps_qe[:], st.x0T[:, kc, :], wqe[:, kc, 0:D],
                                     start=(kc == 0), stop=False)
                nc.tensor.matmul(ps_qe[:], ones1[:], wqe1[0:1, 0:D],
                                 start=False, stop=True)
                for kc in range(DC):
                    nc.tensor.matmul(ps_qe2[:], st.x0T[:, kc, :], wqe[:, kc, D:XW],
                                     start=(kc == 0), stop=False)
                nc.tensor.matmul(ps_qe2[:], ones1[:], wqe1[0:1, D:XW],
                                 start=False, stop=True)
                st.qk_e = work.tile([P, D], BF16, tag="qk_e")
                nc.vector.tensor_copy(st.qk_e[:], ps_qe[:])
                # extras: 0=s64raw 1=qdot_e 2..65=S_pr 66=qdot_r
                st.qd_e = work.tile([P, L + 3], F32, tag="qd_e")
                nc.vector.tensor_copy(st.qd_e[:], ps_qe2[:])
                return st

            def mid_r(ts, st):
                # rel q-chain (mid-stream): qk_r for the r.qk_r dot only;
                # rel scores were folded into the entity q-chain extras
                st.r_sb = r_tiles[ts]
                ps_qr = psum.tile([P, D], F32, tag="ps")
                for kc in range(DC):
                    nc.tensor.matmul(ps_qr[:], st.x0T[:, kc, :], wqr[:, kc, :],
                                     start=(kc == 0), stop=False)
                nc.tensor.matmul(ps_qr[:], ones1[:], wqr1[0:1, :],
                                 start=False, stop=True)
                qk_r = work.tile([P, D], BF16, tag="qk_r")
                nc.vector.tensor_copy(qk_r[:], ps_qr[:])

                junk0 = junkpool.tile([P, D], BF16, tag="junk")
                rdot = work.tile([P, 1], F32, tag="rdot")
                nc.vector.scalar_tensor_tensor(
                    out=junk0[:], in0=st.r_sb[:], scalar=1.0, in1=qk_r[:],
                    op0=ALU.bypass, op1=ALU.mult, accum_out=rdot[:])
                rb = work.tile([P, 1], F32, tag="rb")
                nc.vector.tensor_tensor(out=rb[:], in0=rdot[:],
                                        in1=st.qd_e[:, L + 2:L + 3], op=ALU.add)
                st.p_r = work.tile([P, L], F32, tag="p_r")
                nc.scalar.activation(out=st.p_r[:], in_=st.qd_e[:, 2:L + 2],
                                     func=ACTF.Exp, bias=rb[:, 0:1])

                # C_r^T = pos_r^T @ p_r^T + sar * r^T  (all mid-stream, PE)
                st.sar = work.tile([P, 1], F32, tag="sar")
                nc.vector.tensor_reduce(out=st.sar[:], in_=st.p_r[:], axis=AX.X,
                                        op=ALU.add)
                p_r16 = tailp.tile([P, L], BF16, tag="p_r16")
                nc.vector.tensor_copy(p_r16[:], st.p_r[:])
                ps_prT = psum.tile([L, P], BF16, tag="ps")
                nc.tensor.transpose(ps_prT[:], p_r16[:], ident[:])
                p_rT = tailp.tile([L, P], BF16, tag="p_rT")
                nc.vector.tensor_copy(p_rT[:], ps_prT[:])
                dgr = diagpool.tile([P, P], BF16, tag="dg")
                nc.vector.tensor_scalar(out=dgr[:], in0=ident[:], scalar1=st.sar[:, 0:1],
                                        scalar2=None, op0=ALU.mult)
                st.CrT = tailp.tile([P, DC, P], BF16, tag="CrT")
                for dc in range(DC):
                    ps_cr = psum.tile([P, P], F32, tag="ps", name=f"pscr{ts}_{dc}")
                    nc.tensor.matmul(ps_cr[:], prR[:, dc * P:(dc + 1) * P],
                                     p_rT[:], start=True, stop=False)
                    nc.tensor.matmul(ps_cr[:],
                                     st.r_sb[:, dc * P:(dc + 1) * P], dgr[:],
                                     start=False, stop=True)
                    if dc % 2 == 0:
                        nc.scalar.activation(out=st.CrT[:, dc, :], in_=ps_cr[:],
                                             func=ACTF.Copy)
                    else:
                        nc.vector.tensor_copy(st.CrT[:, dc, :], ps_cr[:])

            def stream(ts, st, hooks):
                # entity scores sx / p; col 64 = mask token (s64 raw + exp bias)
                st.sx = work.tile([P, L + 1], F32, tag="sx")
                nc.vector.tensor_copy(st.sx[:, L:L + 1], st.qd_e[:, 0:1])
                st.p = work.tile([P, L + 1], F32, tag="p")
                st.psCe = []
                for dc in range(DC):
                    ps_ce = psum.tile([P, P], F32, tag=f"ceT{dc}", bufs=1,
                                      name=f"psce{ts}_{dc}")
                    st.psCe.append(ps_ce)
                qb = st.qd_e[:, 1:2]

                def diag_mms(c, xc):
                    for kk in range(KC):
                        k = c * KC + kk
                        dg = diagpool.tile([P, P], BF16, tag="dg")
                        eng = DIAG_PATTERNS[c][kk]
                        if eng == "D":
                            nc.vector.tensor_scalar(
                                out=dg[:], in0=ident[:], scalar1=st.p[:, k:k + 1],
                                scalar2=None, op0=ALU.mult)
                        elif eng == "P":
                            nc.gpsimd.tensor_scalar(
                                out=dg[:], in0=ident[:], scalar1=st.p[:, k:k + 1],
                                scalar2=None, op0=ALU.mult)
                        else:
                            nc.scalar.activation(
                                out=dg[:], in_=ident[:], func=ACTF.Copy,
                                scale=st.p[:, k:k + 1])
                        last = (c == NCHUNK - 1) and (kk == KC - 1)
                        for dc in range(DC):
                            nc.tensor.matmul(
                                st.psCe[dc][:],
                                xc[:, kk, dc * P:(dc + 1) * P], dg[:],
                                start=(k == 0), stop=last)

                xcs = {}

                def fetch(c, ring, nm):
                    xc = xpool.tile([P, KC, D], BF16, tag="xc", name=f"xc{nm}{ts}_{c}")
                    xcs[c] = xc
                    ring.dma_start(xc[:], xp_d[st.rows, c * KC:(c + 1) * KC, :])

                for c in range(NCHUNK):
                    if c not in POOL_CHUNKS:
                        fetch(c, nc.sync, "s")

                def exp_half(c, h):
                    lo = c * KC + h * (KC // 2)
                    nc.scalar.activation(
                        out=st.p[:, lo:lo + KC // 2],
                        in_=st.sx[:, lo:lo + KC // 2],
                        func=ACTF.Exp, bias=qb)

                def dot(c, xc, kk):
                    k = c * KC + kk
                    jt = junkpool.tile([P, D], BF16, tag="junk")
                    if kk < DOT_DVE[c]:
                        nc.vector.scalar_tensor_tensor(
                            out=jt[:], in0=xc[:, kk, :], scalar=1.0,
                            in1=st.qk_e[:], op0=ALU.bypass, op1=ALU.mult,
                            accum_out=st.sx[:, k:k + 1])
                    else:
                        if kk < DOT_DVE[c] + DVE_MULT[c]:
                            nc.vector.tensor_tensor(
                                out=jt[:], in0=xc[:, kk, :], in1=st.qk_e[:],
                                op=ALU.mult)
                        else:
                            nc.gpsimd.tensor_tensor(
                                out=jt[:], in0=xc[:, kk, :], in1=st.qk_e[:],
                                op=ALU.mult)
                        jt2 = junkpool.tile([P, D], BF16, tag="junk")
                        nc.scalar.activation(
                            out=jt2[:], in_=jt[:], func=ACTF.Copy,
                            accum_out=st.sx[:, k:k + 1])

                def diag_mm_one(c, xc, kk):
                    k = c * KC + kk
                    dg = diagpool.tile([P, P], BF16, tag="dg")
                    eng = DIAG_PATTERNS[c][kk]
                    if eng == "D":
                        nc.vector.tensor_scalar(
                            out=dg[:], in0=ident[:], scalar1=st.p[:, k:k + 1],
                            scalar2=None, op0=ALU.mult)
                    elif eng == "P":
                        nc.gpsimd.tensor_scalar(
                            out=dg[:], in0=ident[:], scalar1=st.p[:, k:k + 1],
                            scalar2=None, op0=ALU.mult)
                    else:
                        nc.scalar.activation(
                            out=dg[:], in_=ident[:], func=ACTF.Copy,
                            scale=st.p[:, k:k + 1])
                    last = (c == NCHUNK - 1) and (kk == KC - 1)
                    for dc in range(DC):
                        nc.tensor.matmul(
                            st.psCe[dc][:],
                            xc[:, kk, dc * P:(dc + 1) * P], dg[:],
                            start=(k == 0), stop=last)

                # software-pipelined: chunk c's dots interleave with chunk
                # c-1's exp/diag drain so every engine queue always holds
                # ready work between DMA-blocked instructions
                for c in range(NCHUNK):
                    xc = xcs[c]
                    pc = xcs.get(c - 1) if c > 0 else None
                    if c > 0:
                        exp_half(c - 1, 0)
                    for kk in range(KC):
                        dot(c, xc, kk)
                        if c > 0:
                            if kk == KC // 2 - 1:
                                exp_half(c - 1, 1)
                            diag_mm_one(c - 1, pc, kk)
                    if c + 1 in POOL_CHUNKS:
                        fetch(c + 1, nc.gpsimd, "a")
                    if c == 1:
                        # mask token (early): p64 then C_e^T += Pe64^T x p64
                        nc.scalar.activation(out=st.p[:, L:L + 1],
                                             in_=st.sx[:, L:L + 1],
                                             func=ACTF.Exp, bias=qb)
                        p64b = work.tile([P, 1], BF16, tag="p64b")
                        nc.vector.tensor_copy(p64b[:], st.p[:, L:L + 1])
                        ps_p64 = psum.tile([1, P], BF16, tag="ps")
                        nc.tensor.transpose(ps_p64[:], p64b[:], ident[:])
                        p64T = work.tile([1, P], BF16, tag="p64T")
                        nc.vector.tensor_copy(p64T[:], ps_p64[:])
                        for dc in range(DC):
                            nc.tensor.matmul(
                                st.psCe[dc][:], pe64[0:1, dc * P:(dc + 1) * P],
                                p64T[:], start=False, stop=False)
                    hook = hooks.get(c)
                    if hook is not None:
                        hook()
                exp_half(NCHUNK - 1, 0)
                exp_half(NCHUNK - 1, 1)
                lastxc = xcs.pop(NCHUNK - 1)
                for kk in range(KC):
                    diag_mm_one(NCHUNK - 1, lastxc, kk)

            def tail(ts, st):
                sae = work.tile([P, 1], F32, tag="sae")
                nc.vector.tensor_reduce(out=sae[:], in_=st.p[:], axis=AX.X, op=ALU.add)
                stat_sb = tailp.tile([P, 2], F32, tag="stat_sb")
                nc.vector.tensor_copy(stat_sb[:, 0:1], sae[:])
                nc.vector.tensor_copy(stat_sb[:, 1:2], st.sar[:])
                nc.gpsimd.dma_start(stats_d[st.rows, :], stat_sb[:])

                # C_e^T psum chunks -> SBUF bf16
                CeT = tailp.tile([P, DC, P], BF16, tag="CeT")
                for dc in range(DC):
                    if dc % 2 == 0:
                        nc.vector.tensor_copy(CeT[:, dc, :], st.psCe[dc][:])
                    else:
                        nc.scalar.activation(out=CeT[:, dc, :], in_=st.psCe[dc][:],
                                             func=ACTF.Copy)

                # outT[ec] = sum_dc wev[dc,ec]^T CeT[dc] + wrv[dc,ec]^T CrT[dc]
                ps_oT = psum.tile([P, DC, P], F32, tag="ps")
                for ec in range(DC):
                    for dc in range(DC):
                        nc.tensor.matmul(ps_oT[:, ec, :],
                                         wrv[:, dc, ec * P:(ec + 1) * P],
                                         st.CrT[:, dc, :], start=(dc == 0), stop=False)
                    for dc in range(DC):
                        nc.tensor.matmul(ps_oT[:, ec, :],
                                         wev[:, dc, ec * P:(ec + 1) * P],
                                         CeT[:, dc, :], start=False, stop=(dc == DC - 1))
                o16 = tailp.tile([P, DC, P], BF16, tag="o16", name=f"o16_{ts}")
                for ec in range(DC):
                    if ec % 2 == 0:
                        nc.vector.tensor_copy(o16[:, ec, :], ps_oT[:, ec, :])
                    else:
                        nc.scalar.activation(out=o16[:, ec, :], in_=ps_oT[:, ec, :],
                                             func=ACTF.Copy)
                nc.gpsimd.dma_start(outT_d[ts], o16[:])

            def dma_late():
                nc.sync.dma_start(wev[:], wev_d[:])
                nc.sync.dma_start(wrv[:], wrv_d[:])

            states = []
            for ts in range(NT):
                st = mchain_e(ts)
                mid_r(ts, st)
                states.append(st)
            stream(0, states[0], {1: dma_late})
            tail(0, states[0])
            stream(1, states[1], {})
            tail(1, states[1])

            for _pool in (tailp, diagpool, junkpool, xpool):
                _pool.release()

    nc.finalize()
    return nc


def pack_constants(inputs):
    """Host-side packing of replicated constants (f64 folding, bf16 cast)."""
    import ml_dtypes

    bf16 = ml_dtypes.bfloat16

    def arr(name):
        return np.asarray(inputs[name], dtype=np.float64)

    pos_e = arr("pos_e")
    pos_r = arr("pos_r")[:L]                      # [64, D]
    mask = arr("mask_emb")[0]
    pe64aug = pos_e[L] + mask                     # mask-token embedding (pos incl)

    weq, wek, wrk = arr("We_q"), arr("We_k"), arr("Wr_k")
    beq, bek, brk = arr("be_q"), arr("be_k"), arr("br_k")

    # folded score projections (scale folded in):
    #   qk_e = x0p @ (We_q^T We_k) * scale + (be_q @ We_k) * scale
    wqk_e = (weq.T @ wek) * SCALE
    wqk_r = (weq.T @ wrk) * SCALE
    ue = (beq @ wek) * SCALE
    ur = (beq @ wrk) * SCALE
    # mask-dot column: s64 = x0p.(wqk_e @ pe64aug) + ue.pe64aug (+qdot_e bias)
    v64 = wqk_e @ pe64aug
    c64 = float(ue @ pe64aug)
    # qdot columns: qdot_e = x0p.(We_q^T be_k)*scale + (be_q.be_k)*scale
    vk_e = (weq.T @ bek) * SCALE
    ck_e = float((beq @ bek) * SCALE)
    vk_r = (weq.T @ brk) * SCALE
    ck_r = float((beq @ brk) * SCALE)

    # entity q weights: [D+1 rows, D+2+L+1 cols]
    # cols: [0:D]=qk_e, D=mask-dot, D+1=qdot_e, [D+2:D+2+L]=rel scores, last=qdot_r
    XW = D + 2 + L + 1
    wq_e = np.zeros((D + 1, XW))
    wq_e[:D, :D] = wqk_e
    wq_e[D, :D] = ue
    wq_e[:D, D] = v64
    wq_e[D, D] = c64
    wq_e[:D, D + 1] = vk_e
    wq_e[D, D + 1] = ck_e
    # S_pr[b,k] = x0p[b].(wqk_r @ pos_r[k]) + ur.pos_r[k]
    wq_e[:D, D + 2:D + 2 + L] = wqk_r @ pos_r.T
    wq_e[D, D + 2:D + 2 + L] = ur @ pos_r.T
    wq_e[:D, D + 2 + L] = vk_r
    wq_e[D, D + 2 + L] = ck_r
    wq_r = np.zeros((D + 1, D))
    wq_r[:D, :] = wqk_r
    wq_r[D, :] = ur

    def chunk_rows(w):
        # [D, C] -> [128, DC, C] with (p, dc, j) = w[dc*128+p, j]
        c = w.shape[1]
        return np.ascontiguousarray(w.reshape(DC, P, c))\
            .transpose(1, 0, 2)

    out = {
        "wq_e": chunk_rows(wq_e[:D]).astype(bf16),
        "wq_e1": wq_e[D:D + 1].astype(bf16),
        "wq_r": chunk_rows(wq_r[:D]).astype(bf16),
        "wq_r1": wq_r[D:D + 1].astype(bf16),
        "wev": chunk_rows(np.ascontiguousarray(arr("We_v").T)).astype(bf16),
        "wrv": chunk_rows(np.ascontiguousarray(arr("Wr_v").T)).astype(bf16),
        "prR": pos_r.astype(bf16),
        "pe64": pe64aug[None, :].astype(bf16),
        "ident16": np.eye(P).astype(bf16),
    }
    return {k: np.ascontiguousarray(v) for k, v in out.items()}


def shard_inputs(inputs, core):
    """Per-core input map: bf16 x (pos_e folded), x0 row, r, plus constants."""
    import ml_dtypes

    bf16 = ml_dtypes.bfloat16
    if "_shared" not in _STATE:
        x = np.asarray(inputs["query_entity_encoding"], np.float32)
        pe = np.asarray(inputs["pos_e"], np.float32)[:L]
        xp = (x + pe[None, :, :]).astype(bf16)
        r16 = np.asarray(inputs["relation_encoding"], np.float32).astype(bf16)
        _STATE["_shared"] = (xp, r16, pack_constants(inputs))
    xp, r16, consts = _STATE["_shared"]
    sl = slice(core * BS, (core + 1) * BS)
    m = {"xp16": xp[sl], "x0p16": np.ascontiguousarray(xp[sl, 0, :]),
         "r16": r16[sl]}
    m.update(consts)
    return m


def postprocess(outT, stats, inputs):
    """outT [NT, P, DC, P] bf16, stats [BS, 2] -> out rows [BS, D] f32."""
    o = np.asarray(outT, dtype=np.float32)          # [NT, 128p, 4dc, 128b]
    o = o.transpose(0, 3, 2, 1).reshape(-1, D)      # [BS, D] (d = dc*128+p)
    sae = np.asarray(stats[:, 0], np.float64)[:, None]
    sar = np.asarray(stats[:, 1], np.float64)[:, None]
    be_v = np.asarray(inputs["be_v"], np.float64)[None, :]
    br_v = np.asarray(inputs["br_v"], np.float64)[None, :]
    out = (o + sae * be_v + sar * br_v) / (sae + sar)
    return out.astype(np.float32)


_STATE = {}


def kernel(**inputs):
    if "nc" not in _STATE:
        _STATE["nc"] = build_nc()
    nc = _STATE["nc"]
    _STATE.pop("_shared", None)

    in_maps = [shard_inputs(inputs, i) for i in range(NCORES)]
    res = run_bass_kernel_spmd(nc, in_maps, list(range(NCORES)))
    outs = []
    for i in range(NCORES):
        outs.append(postprocess(res.results[i]["outT"], res.results[i]["stats"],
                                inputs))
    _STATE.pop("_shared", None)
    return np.concatenate(outs, axis=0)
